# revision 32
# baseline (speedup 1.0000x reference)
"""Trainium2 Bass kernel for a dense transformer block (nn_Block_31387620999284).

Sharding: 8 cores = 4 batches x 2 parity groups. Core c handles batch b=c//2
and the query tokens with sequence parity d=c%2 (positions d, d+2, ...). Every
core computes K/V for its batch's full 2048-token sequence (duplicated across
the pair), which removes all cross-core communication. Parity interleaving
makes the causal-attention work identical on every core, so a single NEFF runs
SPMD on all 8 cores with per-core input data only.

On-device layout is "transposed" throughout: [features on partitions, tokens on
free dim]. LayerNorm statistics are computed with ones-vector matmuls on the
tensor engine (partition-dim reduction), then broadcast back across partitions
with gpsimd.partition_broadcast. Matmuls run in bf16 (weights pre-cast on the
host, activations cast on the fly) with fp32 PSUM accumulation; softmax skips
the max-subtraction (scores for this block are bounded by ~3, exp is safe).
The softmax denominator rides along as a 65th ones-column in V, so attention
is exp + mask-multiply + one accumulating matmul chain per (head, q-block).
LN-affine params are folded into the weights/biases on the host.
"""

import sys

for _p in ("/opt/trn_rl_repo",):
    if _p not in sys.path:
        sys.path.append(_p)

import numpy as np
import ml_dtypes
from contextlib import ExitStack

import concourse.bass as bass
import concourse.tile as tile
from concourse import bacc, mybir
from concourse.bass import ts
from concourse.bass_utils import run_bass_kernel_spmd


def _install_ntff_hook():
    """The container's antenv stub lacks axon_hooks; provide it so tracing
    (BASS_TRACE=1) works instead of crashing on import."""
    try:
        import antenv.axon_hooks  # noqa: F401
        return
    except ImportError:
        pass
    try:
        import types
        import antenv
        mod = types.ModuleType("antenv.axon_hooks")
        mod._hook = None
        mod.set_axon_ntff_profile_hook = lambda h: setattr(mod, "_hook", h)
        mod.get_axon_ntff_profile_hook = lambda: mod._hook
        sys.modules["antenv.axon_hooks"] = mod
        antenv.axon_hooks = mod
        try:
            from trn_agent_boot.trn_boot import _ntff_profile_via_ctypes
            mod._hook = _ntff_profile_via_ctypes("/opt/axon/libaxon_pjrt.so")
        except Exception:
            pass
    except Exception:
        pass


_install_ntff_hook()

P = 128
D = 1024
TKV = 2048
TQ = 1024
F = 4096
H = 16
HD = 64
DP = D // P    # 8
FP = F // P    # 32
CH = 512       # token chunk / matmul free dim
QB = 512       # attention query block
NQB = TQ // QB # 2
NKT = TKV // P # 16 key tiles
EPS = 1e-5

F32 = mybir.dt.float32
BF16 = mybir.dt.bfloat16
AF = mybir.ActivationFunctionType


def build_nc():
    nc = bacc.Bacc("TRN2", target_bir_lowering=False, debug=False)

    xT = nc.dram_tensor("xT", [D, TKV], F32, kind="ExternalInput").ap()
    xoT = nc.dram_tensor("xoT", [D, TQ], F32, kind="ExternalInput").ap()
    wq = nc.dram_tensor("wq", [D, D], BF16, kind="ExternalInput").ap()
    wk = nc.dram_tensor("wk", [D, D], BF16, kind="ExternalInput").ap()
    wv = nc.dram_tensor("wv", [D, D], BF16, kind="ExternalInput").ap()
    wo = nc.dram_tensor("wo", [D, D], BF16, kind="ExternalInput").ap()
    w1 = nc.dram_tensor("w1", [D, F], BF16, kind="ExternalInput").ap()
    w2 = nc.dram_tensor("w2", [F, D], BF16, kind="ExternalInput").ap()
    # bias columns: bo 0:8 | b2 8:16 | bq 16:24 | bk 24:32 | b1' 32:64
    biases = nc.dram_tensor("biases", [P, 64], F32, kind="ExternalInput").ap()
    bvr = nc.dram_tensor("bvr", [P, D], F32, kind="ExternalInput").ap()
    mk = nc.dram_tensor("mk", [P, 8, QB], BF16, kind="ExternalInput").ap()
    outT = nc.dram_tensor("outT", [D, TQ], F32, kind="ExternalOutput").ap()

    xT3 = xT.rearrange("(o p) t -> p o t", p=P)
    xoT3 = xoT.rearrange("(o p) t -> p o t", p=P)
    out3 = outT.rearrange("(o p) t -> p o t", p=P)
    wq3 = wq.rearrange("(o p) m -> p o m", p=P)
    wk3 = wk.rearrange("(o p) m -> p o m", p=P)
    wv3 = wv.rearrange("(o p) m -> p o m", p=P)
    wo3 = wo.rearrange("(o p) m -> p o m", p=P)
    w13 = w1.rearrange("(o p) m -> p o m", p=P)
    w23 = w2.rearrange("(o p) m -> p o m", p=P)

    with tile.TileContext(nc) as tc, ExitStack() as ctx:
        consts = ctx.enter_context(tc.tile_pool(name="consts", bufs=1))
        bias_sb = consts.tile([P, 64], F32, name="bias_sb")
        nc.sync.dma_start(bias_sb[:], biases)
        ones_b16 = consts.tile([P, 1], BF16, name="ones_b16")
        nc.vector.memset(ones_b16[:], 1.0)
        ones_f32 = consts.tile([1, P], F32, name="ones_f32")
        nc.vector.memset(ones_f32[:], 1.0)
        eps_sb = consts.tile([P, 1], F32, name="eps_sb")
        nc.vector.memset(eps_sb[:], EPS)

        # ---- LayerNorm (transposed layout) ----
        def make_ln_pools(stack, pfx):
            return dict(
                sq=stack.enter_context(tc.tile_pool(name=pfx + "sq", bufs=2)),
                st=stack.enter_context(tc.tile_pool(name=pfx + "st", bufs=2, space="PSUM")),
                sm=stack.enter_context(tc.tile_pool(name=pfx + "sm", bufs=1)),
                rep=stack.enter_context(tc.tile_pool(name=pfx + "rep", bufs=1, space="PSUM")),
            )

        def ln_norm(lp, get_src, hc):
            """get_src(ks) -> [P, CH] f32 AP; hc: [P, DP, CH] bf16 out.

            Casts x to bf16 into hc, computes mean/var from the bf16 values via
            ones-matmuls, then normalizes hc in place."""
            ps_su = lp["st"].tile([1, CH], F32, name="ps_su", tag="st")
            ps_sq = lp["st"].tile([1, CH], F32, name="ps_sq", tag="st")
            for ks in range(DP):
                src = get_src(ks)
                nc.scalar.copy(hc[:, ks], src)
                sq = lp["sq"].tile([P, CH], BF16, name="sq", tag="sq")
                nc.scalar.activation(sq[:], src, AF.Square)
                nc.tensor.matmul(ps_su[:], ones_b16[:], hc[:, ks],
                                 start=(ks == 0), stop=(ks == DP - 1))
                nc.tensor.matmul(ps_sq[:], ones_b16[:], sq[:],
                                 start=(ks == 0), stop=(ks == DP - 1))
            # r_mu = -mean; r_m2 -> var -> sd -> 1/sd (in place); r_mu -> -mu/sd
            r_mu = lp["sm"].tile([1, CH], F32, name="r_mu", tag="r_mu")
            nc.vector.tensor_scalar_mul(r_mu[:], ps_su[:], -1.0 / D)
            r_m2 = lp["sm"].tile([1, CH], F32, name="r_m2", tag="r_m2")
            nc.vector.tensor_scalar_mul(r_m2[:], ps_sq[:], 1.0 / D)
            mu2 = lp["sm"].tile([1, CH], F32, name="mu2", tag="mu2")
            nc.vector.tensor_mul(mu2[:], r_mu[:], r_mu[:])
            nc.vector.tensor_sub(r_m2[:], r_m2[:], mu2[:])
            nc.scalar.activation(r_m2[:], r_m2[:], AF.Sqrt, bias=eps_sb[0:1])
            nc.vector.reciprocal(r_m2[:], r_m2[:])
            nc.vector.tensor_mul(r_mu[:], r_mu[:], r_m2[:])
            # replicate the two stat rows across partitions on the PE
            repA = lp["rep"].tile([P, CH], F32, name="repA", tag="repA")
            nc.tensor.matmul(repA[:], ones_f32[:], r_m2[:], start=True, stop=True)
            repB = lp["rep"].tile([P, CH], F32, name="repB", tag="repB")
            nc.tensor.matmul(repB[:], ones_f32[:], r_mu[:], start=True, stop=True)
            for ks in range(DP):
                nc.vector.tensor_mul(hc[:, ks], hc[:, ks], repA[:])
                nc.vector.tensor_add(hc[:, ks], hc[:, ks], repB[:])

        # Persistent K/V/Q for attention (phases 1-2).
        sKVQ = ExitStack()
        kvqp = sKVQ.enter_context(tc.tile_pool(name="kvqp", bufs=1))
        KT_all = kvqp.tile([P, DP, TKV], BF16, name="KT_all")
        V_all = kvqp.tile([P, NKT, H, HD + 1], BF16, name="V_all")
        QT_all = kvqp.tile([P, DP, TQ], BF16, name="QT_all")

        # Pools that must outlive phase transitions sit on the right side so
        # their DMAs never alias freed left-side addresses (no false deps).
        sWX = ExitStack()
        mskp = sWX.enter_context(tc.tile_pool(name="mskp", bufs=1, side="right"))
        mask_sb = mskp.tile([P, 8, QB], BF16, name="mask_sb")
        nc.sync.dma_start(mask_sb[:], mk)
        wop = sWX.enter_context(tc.tile_pool(name="wop", bufs=1, side="right"))
        wo_sb = wop.tile([P, DP, D], BF16, name="wo_sb")
        xop = sWX.enter_context(tc.tile_pool(name="xop", bufs=3, side="right"))

        # ================= Phase 1: LN1 + Q/K/V projections =================
        with ExitStack() as p1:
            lp1 = make_ln_pools(p1, "l1")
            xcp = p1.enter_context(tc.tile_pool(name="xcp", bufs=4))
            hcp = p1.enter_context(tc.tile_pool(name="hcp", bufs=2))
            mmp = p1.enter_context(tc.tile_pool(name="mmp1", bufs=3, space="PSUM"))
            bvp = p1.enter_context(tc.tile_pool(name="bvp", bufs=1))
            wkvp = p1.enter_context(tc.tile_pool(name="wkvp", bufs=1))
            wqp = p1.enter_context(tc.tile_pool(name="wqp", bufs=1, side="right"))

            # chunk list: 4 KV chunks then 2 Q chunks, software-pipelined so the
            # LN chain of chunk i+1 overlaps the projection matmuls of chunk i.
            chunks = [("kv", c) for c in range(TKV // CH)] + \
                     [("q", c) for c in range(TQ // CH)]
            hcs = {}

            def load_ln(idx):
                kind, c = chunks[idx]
                src3 = xT3 if kind == "kv" else xoT3

                def get_src(ks, src3=src3, c=c):
                    xk = xcp.tile([P, CH], F32, name="xk", tag="xk")
                    nc.sync.dma_start(xk[:], src3[:, ks, ts(c, CH)])
                    return xk[:]

                hc = hcp.tile([P, DP, CH], BF16, name="hc", tag="hc")
                ln_norm(lp1, get_src, hc)
                hcs[idx] = hc

            load_ln(0)

            # weight/bias loads traced after the first chunk's LN so the PE can
            # start on stats immediately; DMAs overlap the LN chain.
            nc.vector.memset(V_all[:, :, :, HD:HD + 1], 1.0)
            bvr_sb = bvp.tile([P, D], F32, name="bvr_sb")
            nc.sync.dma_start(bvr_sb[:], bvr)
            wk_sb = wkvp.tile([P, DP, D], BF16, name="wk_sb")
            nc.sync.dma_start(wk_sb[:], wk3)
            wv_sb = wkvp.tile([P, DP, D], BF16, name="wv_sb")
            nc.sync.dma_start(wv_sb[:], wv3)
            wq_sb = wqp.tile([P, DP, D], BF16, name="wq_sb")
            nc.sync.dma_start(wq_sb[:], wq3)
            nc.sync.dma_start(wo_sb[:], wo3)

            for idx, (kind, c) in enumerate(chunks):
                if idx + 1 < len(chunks):
                    load_ln(idx + 1)
                hc = hcs.pop(idx)
                if kind == "kv":
                    for hp in range(DP):
                        ps = mmp.tile([P, CH], F32, name="psk", tag="mm1")
                        for ks in range(DP):
                            nc.tensor.matmul(ps[:], wk_sb[:, ks, ts(hp, P)], hc[:, ks],
                                             start=(ks == 0), stop=(ks == DP - 1))
                        nc.vector.tensor_scalar_add(KT_all[:, hp, ts(c, CH)], ps[:],
                                                    bias_sb[:, 24 + hp:25 + hp])
                    for dc in range(2):
                        for st in range(4):
                            ps = mmp.tile([P, CH], F32, name="psv", tag="mm1")
                            for ks in range(DP):
                                nc.tensor.matmul(ps[:], hc[:, ks, ts(st, P)],
                                                 wv_sb[:, ks, ts(dc, CH)],
                                                 start=(ks == 0), stop=(ks == DP - 1))
                            vdst = V_all[:, c * 4 + st, dc * 8:dc * 8 + 8, 0:HD]
                            nc.vector.tensor_add(
                                vdst,
                                ps[:].rearrange("p (h d) -> p h d", h=8),
                                bvr_sb[:, ts(dc, CH)].rearrange("p (h d) -> p h d", h=8))
                else:
                    for hp in range(DP):
                        ps = mmp.tile([P, CH], F32, name="psq", tag="mm1")
                        for ks in range(DP):
                            nc.tensor.matmul(ps[:], wq_sb[:, ks, ts(hp, P)], hc[:, ks],
                                             start=(ks == 0), stop=(ks == DP - 1))
                        nc.vector.tensor_scalar_add(QT_all[:, hp, ts(c, CH)], ps[:],
                                                    bias_sb[:, 16 + hp:17 + hp])

        # ================= Phase 2: causal attention =================
        sATT = ExitStack()
        attp = sATT.enter_context(tc.tile_pool(name="attp", bufs=1, side="right"))
        attn_all = attp.tile([P, DP, TQ], BF16, name="attn_all")
        with ExitStack() as p2:
            psS = p2.enter_context(tc.tile_pool(name="psS", bufs=4, space="PSUM"))
            psAV = p2.enter_context(tc.tile_pool(name="psAV", bufs=1, space="PSUM"))
            psR = p2.enter_context(tc.tile_pool(name="psR", bufs=1, space="PSUM"))
            weip = p2.enter_context(tc.tile_pool(name="weip", bufs=4))
            smal = p2.enter_context(tc.tile_pool(name="smal", bufs=4))

            scale = float(HD) ** -0.5
            for t in range(NQB):           # q-blocks outermost: lets phase 3's
                nkt = 8 * (t + 1)          # first chunk start after t=0 is done
                for hp in range(DP):
                    pavs = [psAV.tile([65, QB], F32, name=f"pav{l}", tag=f"pav{l}")
                            for l in range(2)]
                    weis = {}

                    def scores(l, kt, t=t, hp=hp, weis=weis):
                        pb = 64 * l
                        ps = psS.tile([P, QB], F32, name="pss", tag="pss")
                        nc.tensor.matmul(ps[:],
                                         KT_all[pb:pb + 64, hp, ts(kt, P)],
                                         QT_all[pb:pb + 64, hp, ts(t, QB)],
                                         start=True, stop=True)
                        wei = weip.tile([P, QB], BF16, name="wei", tag="wei")
                        nc.scalar.activation(wei[:], ps[:], AF.Exp, scale=scale)
                        if kt >= 8 * t:
                            nc.vector.tensor_mul(wei[:], wei[:],
                                                 mask_sb[:, kt - 8 * t, :])
                        weis[(l, kt)] = wei

                    scores(0, 0)
                    scores(1, 0)
                    for kt in range(nkt):
                        if kt + 1 < nkt:
                            scores(0, kt + 1)
                            scores(1, kt + 1)
                        for l in range(2):
                            nc.tensor.matmul(pavs[l][:],
                                             V_all[:, kt, 2 * hp + l, :],
                                             weis.pop((l, kt)),
                                             start=(kt == 0), stop=(kt == nkt - 1))
                    for l in range(2):
                        pb = 64 * l
                        den = smal.tile([1, QB], F32, name="den", tag="den")
                        nc.scalar.copy(den[:], pavs[l][64:65, :])
                        prep = psR.tile([64, QB], F32, name="prep", tag="prep")
                        nc.tensor.matmul(prep[:], ones_f32[:, 0:64], den[:],
                                         start=True, stop=True)
                        rec = smal.tile([64, QB], F32, name="rec", tag="rec")
                        nc.vector.reciprocal(rec[:], prep[:])
                        nc.vector.tensor_mul(attn_all[pb:pb + 64, hp, ts(t, QB)],
                                             pavs[l][0:64, :], rec[:])
        sKVQ.close()

        # ================= Phase 3: output proj + residual + LN2 =================
        sX2 = ExitStack()
        x2p = sX2.enter_context(tc.tile_pool(name="x2p", bufs=1))
        x2T = x2p.tile([P, DP, TQ], F32, name="x2T")
        h2T = x2p.tile([P, DP, TQ], BF16, name="h2T")
        with ExitStack() as p3:
            lp3 = make_ln_pools(p3, "l3")
            ps3 = p3.enter_context(tc.tile_pool(name="ps3", bufs=3, space="PSUM"))
            tp3 = p3.enter_context(tc.tile_pool(name="tp3", bufs=4))
            for qc in range(TQ // CH):
                for i in range(DP):
                    xo = xop.tile([P, CH], F32, name="xo", tag="xo")
                    nc.sync.dma_start(xo[:], xoT3[:, i, ts(qc, CH)])
                    ps = ps3.tile([P, CH], F32, name="pso", tag="pso")
                    for ks in range(DP):
                        nc.tensor.matmul(ps[:], wo_sb[:, ks, ts(i, P)],
                                         attn_all[:, ks, ts(qc, CH)],
                                         start=(ks == 0), stop=(ks == DP - 1))
                    t1 = tp3.tile([P, CH], F32, name="t1", tag="t1")
                    nc.vector.tensor_add(t1[:], ps[:], xo[:])
                    nc.vector.tensor_scalar_add(x2T[:, i, ts(qc, CH)], t1[:],
                                                bias_sb[:, 0 + i:1 + i])
                # LN2 for this chunk immediately; overlaps next chunk's matmuls
                ln_norm(lp3, lambda ks, qc=qc: x2T[:, ks, ts(qc, CH)],
                        h2T[:, :, ts(qc, CH)])
        sATT.close()
        sWX.close()

        # ================= Phase 4: FFN + residual =================
        with ExitStack() as p4:
            w1p = p4.enter_context(tc.tile_pool(name="w1p", bufs=2))
            w2p = p4.enter_context(tc.tile_pool(name="w2p", bufs=2))
            rp = p4.enter_context(tc.tile_pool(name="rp", bufs=2))
            psF = p4.enter_context(tc.tile_pool(name="psF", bufs=4, space="PSUM"))
            psO = p4.enter_context(tc.tile_pool(name="psO", bufs=2, space="PSUM"))
            top = p4.enter_context(tc.tile_pool(name="top", bufs=4))
            for qc in range(TQ // CH):
                rT = rp.tile([P, FP, CH], BF16, name="rT", tag="rT")
                for fs in range(8):
                    w1c = w1p.tile([P, DP, CH], BF16, name="w1c", tag="w1c")
                    nc.sync.dma_start(w1c[:], w13[:, :, ts(fs, CH)])
                    for fj in range(4):
                        f = fs * 4 + fj
                        ps = psF.tile([P, CH], F32, name="psf", tag="psf")
                        for ks in range(DP):
                            nc.tensor.matmul(ps[:], w1c[:, ks, ts(fj, P)],
                                             h2T[:, ks, ts(qc, CH)],
                                             start=(ks == 0), stop=(ks == DP - 1))
                        nc.scalar.activation(rT[:, f], ps[:], AF.Relu,
                                             bias=bias_sb[:, 32 + f:33 + f])
                for i in range(DP):
                    w2i = w2p.tile([P, FP, P], BF16, name="w2i", tag="w2i")
                    nc.sync.dma_start(w2i[:], w23[:, :, ts(i, P)])
                    ps2 = psO.tile([P, CH], F32, name="ps2", tag="ps2")
                    for f in range(FP):
                        nc.tensor.matmul(ps2[:], w2i[:, f, :], rT[:, f],
                                         start=(f == 0), stop=(f == FP - 1))
                    t2 = top.tile([P, CH], F32, name="t2", tag="t2")
                    nc.vector.tensor_add(t2[:], ps2[:], x2T[:, i, ts(qc, CH)])
                    ot = top.tile([P, CH], F32, name="ot", tag="ot")
                    nc.vector.tensor_scalar_add(ot[:], t2[:], bias_sb[:, 8 + i:9 + i])
                    nc.sync.dma_start(out3[:, i, ts(qc, CH)], ot[:])
        sX2.close()

    nc.compile()
    return nc


def prepare_inputs(x, wq, wk, wv, wo, bo, w1, b1, w2, b2,
                   g_ln1, b_ln1, g_ln2, b_ln2):
    """Host-side sharding/prep. Returns list of 8 per-core input dicts."""
    f32 = np.float32
    bf = ml_dtypes.bfloat16
    x = np.asarray(x, f32)
    g1 = np.asarray(g_ln1, f32)
    b1n = np.asarray(b_ln1, f32)
    g2 = np.asarray(g_ln2, f32)
    b2n = np.asarray(b_ln2, f32)

    wq_e = np.ascontiguousarray((g1[:, None] * np.asarray(wq, f32)).astype(bf))
    wk_e = np.ascontiguousarray((g1[:, None] * np.asarray(wk, f32)).astype(bf))
    wv_e = np.ascontiguousarray((g1[:, None] * np.asarray(wv, f32)).astype(bf))
    wo_e = np.ascontiguousarray(np.asarray(wo, f32).astype(bf))
    w1_e = np.ascontiguousarray((g2[:, None] * np.asarray(w1, f32)).astype(bf))
    w2_e = np.ascontiguousarray(np.asarray(w2, f32).astype(bf))

    bq = b1n @ np.asarray(wq, f32)
    bk = b1n @ np.asarray(wk, f32)
    bv = b1n @ np.asarray(wv, f32)
    b1p = np.asarray(b1, f32) + b2n @ np.asarray(w1, f32)

    def pcol(v, n):  # [n*128] -> [128, n] partition-major
        return np.ascontiguousarray(np.asarray(v, f32).reshape(n, P).T)

    biases = np.zeros((P, 64), f32)
    biases[:, 0:8] = pcol(bo, 8)
    biases[:, 8:16] = pcol(b2, 8)
    biases[:, 16:24] = pcol(bq, 8)
    biases[:, 24:32] = pcol(bk, 8)
    biases[:, 32:64] = pcol(b1p, 32)
    bvr = np.ascontiguousarray(np.broadcast_to(bv[None, :], (P, D)))

    masks = {}
    for d in (0, 1):
        p = np.arange(P)[:, None, None]
        j = np.arange(8)[None, :, None]
        qq = np.arange(QB)[None, None, :]
        masks[d] = np.ascontiguousarray(
            ((128 * j + p) <= (2 * qq + d)).astype(bf))

    in_maps = []
    for c in range(8):
        b, d = divmod(c, 2)
        in_maps.append(dict(
            xT=np.ascontiguousarray(x[b].T),
            xoT=np.ascontiguousarray(x[b, d::2].T),
            wq=wq_e, wk=wk_e, wv=wv_e, wo=wo_e, w1=w1_e, w2=w2_e,
            biases=biases, bvr=bvr, mk=masks[d],
        ))
    return in_maps


_NC = None
LAST_RESULTS = None


def kernel(**inputs):
    global _NC, LAST_RESULTS
    in_maps = prepare_inputs(**inputs)
    if _NC is None:
        _NC = build_nc()
    res = run_bass_kernel_spmd(_NC, in_maps, core_ids=list(range(8)))
    LAST_RESULTS = res
    out = np.empty((4, TKV, D), np.float32)
    for c in range(8):
        b, d = divmod(c, 2)
        out[b, d::2, :] = res.results[c]["outT"].T
    return out


if __name__ == "__main__":
    z = np.load("/root/problem/ref_cache.npz")
    inputs = {k: z[k] for k in z.files if k != "out"}
    out = kernel(**inputs)
    ref = z["out"]
    err = np.abs(out - ref)
    print("abs max err:", err.max(), "scale-rel:", err.max() / np.abs(ref).max())


# revision 36
# speedup vs baseline: 1.2033x; 1.2033x over previous
"""Trainium2 Bass kernel for a dense transformer block (nn_Block_31387620999284).

Sharding: 8 cores = 4 batches x 2 parity groups. Core c handles batch b=c//2
and the query tokens with sequence parity d=c%2 (positions d, d+2, ...). Every
core computes K/V for its batch's full 2048-token sequence (duplicated across
the pair), which removes all cross-core communication. Parity interleaving
makes the causal-attention work identical on every core, so a single NEFF runs
SPMD on all 8 cores with per-core input data only.

On-device layout is "transposed" throughout: [features on partitions, tokens on
free dim]. LayerNorm statistics are computed with ones-vector matmuls on the
tensor engine (partition-dim reduction), then broadcast back across partitions
with gpsimd.partition_broadcast. Matmuls run in bf16 (weights pre-cast on the
host, activations cast on the fly) with fp32 PSUM accumulation; softmax skips
the max-subtraction (scores for this block are bounded by ~3, exp is safe).
The softmax denominator rides along as a 65th ones-column in V, so attention
is exp + mask-multiply + one accumulating matmul chain per (head, q-block).
LN-affine params are folded into the weights/biases on the host.
"""

import sys

for _p in ("/opt/trn_rl_repo",):
    if _p not in sys.path:
        sys.path.append(_p)

import numpy as np
import ml_dtypes
from contextlib import ExitStack

import concourse.bass as bass
import concourse.tile as tile
from concourse import bacc, mybir
from concourse.bass import ts
from concourse.bass_utils import run_bass_kernel_spmd


def _install_ntff_hook():
    """The container's antenv stub lacks axon_hooks; provide it so tracing
    (BASS_TRACE=1) works instead of crashing on import."""
    try:
        import antenv.axon_hooks  # noqa: F401
        return
    except ImportError:
        pass
    try:
        import types
        import antenv
        mod = types.ModuleType("antenv.axon_hooks")
        mod._hook = None
        mod.set_axon_ntff_profile_hook = lambda h: setattr(mod, "_hook", h)
        mod.get_axon_ntff_profile_hook = lambda: mod._hook
        sys.modules["antenv.axon_hooks"] = mod
        antenv.axon_hooks = mod
        try:
            from trn_agent_boot.trn_boot import _ntff_profile_via_ctypes
            mod._hook = _ntff_profile_via_ctypes("/opt/axon/libaxon_pjrt.so")
        except Exception:
            pass
    except Exception:
        pass


_install_ntff_hook()

P = 128
D = 1024
TKV = 2048
TQ = 1024
F = 4096
H = 16
HD = 64
DP = D // P    # 8
FP = F // P    # 32
CH = 512       # token chunk / matmul free dim
QB = 512       # attention query block
NQB = TQ // QB # 2
NKT = TKV // P # 16 key tiles
EPS = 1e-5

F32 = mybir.dt.float32
BF16 = mybir.dt.bfloat16
AF = mybir.ActivationFunctionType


def build_nc():
    nc = bacc.Bacc("TRN2", target_bir_lowering=False, debug=False)

    xT = nc.dram_tensor("xT", [D, TKV], F32, kind="ExternalInput").ap()
    xoT = nc.dram_tensor("xoT", [D, TQ], F32, kind="ExternalInput").ap()
    wq = nc.dram_tensor("wq", [D, D], BF16, kind="ExternalInput").ap()
    wk = nc.dram_tensor("wk", [D, D], BF16, kind="ExternalInput").ap()
    wv = nc.dram_tensor("wv", [D, D], BF16, kind="ExternalInput").ap()
    wo = nc.dram_tensor("wo", [D, D], BF16, kind="ExternalInput").ap()
    w1 = nc.dram_tensor("w1", [D, F], BF16, kind="ExternalInput").ap()
    w2 = nc.dram_tensor("w2", [F, D], BF16, kind="ExternalInput").ap()
    # bias columns: bo 0:8 | b2 8:16 | bq 16:24 | bk 24:32 | b1' 32:64
    biases = nc.dram_tensor("biases", [P, 64], F32, kind="ExternalInput").ap()
    bvr = nc.dram_tensor("bvr", [P, D], F32, kind="ExternalInput").ap()
    mk = nc.dram_tensor("mk", [P, 64], BF16, kind="ExternalInput").ap()
    outT = nc.dram_tensor("outT", [D, TQ], F32, kind="ExternalOutput").ap()

    xT3 = xT.rearrange("(o p) t -> p o t", p=P)
    xoT3 = xoT.rearrange("(o p) t -> p o t", p=P)
    out3 = outT.rearrange("(o p) t -> p o t", p=P)
    wq3 = wq.rearrange("(o p) m -> p o m", p=P)
    wk3 = wk.rearrange("(o p) m -> p o m", p=P)
    wv3 = wv.rearrange("(o p) m -> p o m", p=P)
    wo3 = wo.rearrange("(o p) m -> p o m", p=P)
    w13 = w1.rearrange("(o p) m -> p o m", p=P)
    w23 = w2.rearrange("(o p) m -> p o m", p=P)

    with tile.TileContext(nc) as tc, ExitStack() as ctx:
        consts = ctx.enter_context(tc.tile_pool(name="consts", bufs=1))
        bias_sb = consts.tile([P, 64], F32, name="bias_sb")
        nc.sync.dma_start(bias_sb[:], biases)
        ones_b16 = consts.tile([P, 1], BF16, name="ones_b16")
        nc.vector.memset(ones_b16[:], 1.0)
        ones_f32 = consts.tile([1, P], F32, name="ones_f32")
        nc.vector.memset(ones_f32[:], 1.0)
        eps_sb = consts.tile([P, 1], F32, name="eps_sb")
        nc.vector.memset(eps_sb[:], EPS)

        # ---- LayerNorm (transposed layout) ----
        def make_ln_pools(stack, pfx):
            return dict(
                sq=stack.enter_context(tc.tile_pool(name=pfx + "sq", bufs=2)),
                st=stack.enter_context(tc.tile_pool(name=pfx + "st", bufs=2, space="PSUM")),
                sm=stack.enter_context(tc.tile_pool(name=pfx + "sm", bufs=1)),
                rep=stack.enter_context(tc.tile_pool(name=pfx + "rep", bufs=1, space="PSUM")),
            )

        def ln_norm(lp, get_src, hc):
            """get_src(ks) -> [P, CH] f32 AP; hc: [P, DP, CH] bf16 out.

            Casts x to bf16 into hc, computes mean/var from the bf16 values via
            ones-matmuls, then normalizes hc in place."""
            ps_su = lp["st"].tile([1, CH], F32, name="ps_su", tag="st")
            ps_sq = lp["st"].tile([1, CH], F32, name="ps_sq", tag="st")
            for ks in range(DP):
                src = get_src(ks)
                nc.scalar.copy(hc[:, ks], src)
                sq = lp["sq"].tile([P, CH], BF16, name="sq", tag="sq")
                nc.scalar.activation(sq[:], src, AF.Square)
                nc.tensor.matmul(ps_su[:], ones_b16[:], hc[:, ks],
                                 start=(ks == 0), stop=(ks == DP - 1))
                nc.tensor.matmul(ps_sq[:], ones_b16[:], sq[:],
                                 start=(ks == 0), stop=(ks == DP - 1))
            # r_mu = -mean; r_m2 -> var -> sd -> 1/sd (in place); r_mu -> -mu/sd
            r_mu = lp["sm"].tile([1, CH], F32, name="r_mu", tag="r_mu")
            nc.vector.tensor_scalar_mul(r_mu[:], ps_su[:], -1.0 / D)
            r_m2 = lp["sm"].tile([1, CH], F32, name="r_m2", tag="r_m2")
            nc.vector.tensor_scalar_mul(r_m2[:], ps_sq[:], 1.0 / D)
            mu2 = lp["sm"].tile([1, CH], F32, name="mu2", tag="mu2")
            nc.vector.tensor_mul(mu2[:], r_mu[:], r_mu[:])
            nc.vector.tensor_sub(r_m2[:], r_m2[:], mu2[:])
            nc.scalar.activation(r_m2[:], r_m2[:], AF.Sqrt, bias=eps_sb[0:1])
            nc.vector.reciprocal(r_m2[:], r_m2[:])
            nc.vector.tensor_mul(r_mu[:], r_mu[:], r_m2[:])
            # replicate the two stat rows across partitions on the PE
            repA = lp["rep"].tile([P, CH], F32, name="repA", tag="repA")
            nc.tensor.matmul(repA[:], ones_f32[:], r_m2[:], start=True, stop=True)
            repB = lp["rep"].tile([P, CH], F32, name="repB", tag="repB")
            nc.tensor.matmul(repB[:], ones_f32[:], r_mu[:], start=True, stop=True)
            for ks in range(DP):
                nc.vector.tensor_mul(hc[:, ks], hc[:, ks], repA[:])
                nc.vector.tensor_add(hc[:, ks], hc[:, ks], repB[:])

        # Persistent K/V/Q for attention (phases 1-2).
        sKVQ = ExitStack()
        kvqp = sKVQ.enter_context(tc.tile_pool(name="kvqp", bufs=1))
        KT_all = kvqp.tile([P, DP, TKV], BF16, name="KT_all")
        V_all = kvqp.tile([P, NKT, H, HD + 1], BF16, name="V_all")
        QT_all = kvqp.tile([P, DP, TQ], BF16, name="QT_all")

        # Pools that must outlive phase transitions sit on the right side so
        # their DMAs never alias freed left-side addresses (no false deps).
        sWX = ExitStack()
        mskp = sWX.enter_context(tc.tile_pool(name="mskp", bufs=1, side="right"))
        mask_sb = mskp.tile([P, 64], BF16, name="mask_sb")
        nc.sync.dma_start(mask_sb[:], mk)
        wop = sWX.enter_context(tc.tile_pool(name="wop", bufs=1, side="right"))
        wo_sb = wop.tile([P, DP, D], BF16, name="wo_sb")
        xop = sWX.enter_context(tc.tile_pool(name="xop", bufs=3, side="right"))

        # ================= Phase 1: LN1 + Q/K/V projections =================
        with ExitStack() as p1:
            lp1 = make_ln_pools(p1, "l1")
            xcp = p1.enter_context(tc.tile_pool(name="xcp", bufs=4))
            hcp = p1.enter_context(tc.tile_pool(name="hcp", bufs=2))
            mmp = p1.enter_context(tc.tile_pool(name="mmp1", bufs=3, space="PSUM"))
            bvp = p1.enter_context(tc.tile_pool(name="bvp", bufs=1))
            wkvp = p1.enter_context(tc.tile_pool(name="wkvp", bufs=1))
            wqp = p1.enter_context(tc.tile_pool(name="wqp", bufs=1, side="right"))

            # chunk list: 4 KV chunks then 2 Q chunks, software-pipelined so the
            # LN chain of chunk i+1 overlaps the projection matmuls of chunk i.
            chunks = [("kv", c) for c in range(TKV // CH)] + \
                     [("q", c) for c in range(TQ // CH)]
            hcs = {}

            def load_ln(idx):
                kind, c = chunks[idx]
                src3 = xT3 if kind == "kv" else xoT3

                def get_src(ks, src3=src3, c=c):
                    xk = xcp.tile([P, CH], F32, name="xk", tag="xk")
                    nc.sync.dma_start(xk[:], src3[:, ks, ts(c, CH)])
                    return xk[:]

                hc = hcp.tile([P, DP, CH], BF16, name="hc", tag="hc")
                ln_norm(lp1, get_src, hc)
                hcs[idx] = hc

            load_ln(0)

            # weight/bias loads traced after the first chunk's LN so the PE can
            # start on stats immediately; DMAs overlap the LN chain.
            nc.vector.memset(V_all[:, :, :, HD:HD + 1], 1.0)
            bvr_sb = bvp.tile([P, D], F32, name="bvr_sb")
            nc.sync.dma_start(bvr_sb[:], bvr)
            wk_sb = wkvp.tile([P, DP, D], BF16, name="wk_sb")
            nc.sync.dma_start(wk_sb[:], wk3)
            wv_sb = wkvp.tile([P, DP, D], BF16, name="wv_sb")
            nc.sync.dma_start(wv_sb[:], wv3)
            wq_sb = wqp.tile([P, DP, D], BF16, name="wq_sb")
            nc.sync.dma_start(wq_sb[:], wq3)
            nc.sync.dma_start(wo_sb[:], wo3)

            for idx, (kind, c) in enumerate(chunks):
                if idx + 1 < len(chunks):
                    load_ln(idx + 1)
                hc = hcs.pop(idx)
                if kind == "kv":
                    for hp in range(DP):
                        ps = mmp.tile([P, CH], F32, name="psk", tag="mm1")
                        for ks in range(DP):
                            nc.tensor.matmul(ps[:], wk_sb[:, ks, ts(hp, P)], hc[:, ks],
                                             start=(ks == 0), stop=(ks == DP - 1))
                        nc.vector.tensor_scalar_add(KT_all[:, hp, ts(c, CH)], ps[:],
                                                    bias_sb[:, 24 + hp:25 + hp])
                    for dc in range(2):
                        for st in range(4):
                            ps = mmp.tile([P, CH], F32, name="psv", tag="mm1")
                            for ks in range(DP):
                                nc.tensor.matmul(ps[:], hc[:, ks, ts(st, P)],
                                                 wv_sb[:, ks, ts(dc, CH)],
                                                 start=(ks == 0), stop=(ks == DP - 1))
                            vdst = V_all[:, c * 4 + st, dc * 8:dc * 8 + 8, 0:HD]
                            nc.vector.tensor_add(
                                vdst,
                                ps[:].rearrange("p (h d) -> p h d", h=8),
                                bvr_sb[:, ts(dc, CH)].rearrange("p (h d) -> p h d", h=8))
                else:
                    for hp in range(DP):
                        ps = mmp.tile([P, CH], F32, name="psq", tag="mm1")
                        for ks in range(DP):
                            nc.tensor.matmul(ps[:], wq_sb[:, ks, ts(hp, P)], hc[:, ks],
                                             start=(ks == 0), stop=(ks == DP - 1))
                        nc.vector.tensor_scalar_add(QT_all[:, hp, ts(c, CH)], ps[:],
                                                    bias_sb[:, 16 + hp:17 + hp])

        # ================= Phase 2: causal attention =================
        sATT = ExitStack()
        attp = sATT.enter_context(tc.tile_pool(name="attp", bufs=1, side="right"))
        attn_all = attp.tile([P, DP, TQ], BF16, name="attn_all")
        with ExitStack() as p2:
            psS = p2.enter_context(tc.tile_pool(name="psS", bufs=3, space="PSUM"))
            psAV = p2.enter_context(tc.tile_pool(name="psAV", bufs=2, space="PSUM"))
            psR = p2.enter_context(tc.tile_pool(name="psR", bufs=1, space="PSUM"))
            weip = p2.enter_context(tc.tile_pool(name="weip", bufs=4))
            smal = p2.enter_context(tc.tile_pool(name="smal", bufs=4))

            scale = float(HD) ** -0.5
            for t in range(NQB):           # q-blocks outermost: lets phase 3's
                nkt = 8 * (t + 1)          # first chunk start after t=0 is done
                for hp in range(DP):
                    pavs = [psAV.tile([65, QB], F32, name=f"pav{l}", tag=f"pav{l}")
                            for l in range(2)]
                    weis = {}

                    # Causal structure: for q-block t, key tile kt = 8*t + j is
                    # "diagonal": columns < 64*j are fully masked (skipped
                    # entirely), columns [64j, 64j+64) need the (j-independent)
                    # stride-2 mask, the rest are fully allowed.
                    def col0(kt, t=t):
                        return 64 * (kt - 8 * t) if kt >= 8 * t else 0

                    def scores(l, kt, t=t, hp=hp, weis=weis):
                        pb = 64 * l
                        o = col0(kt)
                        ps = psS.tile([P, QB], F32, name="pss", tag="pss")
                        nc.tensor.matmul(ps[:, o:],
                                         KT_all[pb:pb + 64, hp, ts(kt, P)],
                                         QT_all[pb:pb + 64, hp,
                                                t * QB + o:(t + 1) * QB],
                                         start=True, stop=True)
                        wei = weip.tile([P, QB], BF16, name="wei", tag="wei")
                        nc.scalar.activation(wei[:, o:], ps[:, o:], AF.Exp,
                                             scale=scale)
                        if kt >= 8 * t:
                            nc.vector.tensor_mul(wei[:, o:o + 64], wei[:, o:o + 64],
                                                 mask_sb[:])
                        weis[(l, kt)] = wei

                    scores(0, 0)
                    scores(1, 0)
                    for kt in range(nkt):
                        if kt + 1 < nkt:
                            scores(0, kt + 1)
                            scores(1, kt + 1)
                        o = col0(kt)
                        for l in range(2):
                            nc.tensor.matmul(pavs[l][:, o:],
                                             V_all[:, kt, 2 * hp + l, :],
                                             weis.pop((l, kt))[:, o:],
                                             start=(kt == 0), stop=(kt == nkt - 1))
                    for l in range(2):
                        pb = 64 * l
                        den = smal.tile([1, QB], F32, name="den", tag="den")
                        nc.scalar.copy(den[:], pavs[l][64:65, :])
                        prep = psR.tile([64, QB], F32, name="prep", tag="prep")
                        nc.tensor.matmul(prep[:], ones_f32[:, 0:64], den[:],
                                         start=True, stop=True)
                        rec = smal.tile([64, QB], F32, name="rec", tag="rec")
                        nc.vector.reciprocal(rec[:], prep[:])
                        nc.vector.tensor_mul(attn_all[pb:pb + 64, hp, ts(t, QB)],
                                             pavs[l][0:64, :], rec[:])
        sKVQ.close()

        # ================= Phase 3: output proj + residual + LN2 =================
        sX2 = ExitStack()
        x2p = sX2.enter_context(tc.tile_pool(name="x2p", bufs=1))
        x2T = x2p.tile([P, DP, TQ], F32, name="x2T")
        h2T = x2p.tile([P, DP, TQ], BF16, name="h2T")
        with ExitStack() as p3:
            lp3 = make_ln_pools(p3, "l3")
            ps3 = p3.enter_context(tc.tile_pool(name="ps3", bufs=3, space="PSUM"))
            tp3 = p3.enter_context(tc.tile_pool(name="tp3", bufs=4))
            for qc in range(TQ // CH):
                for i in range(DP):
                    xo = xop.tile([P, CH], F32, name="xo", tag="xo")
                    nc.sync.dma_start(xo[:], xoT3[:, i, ts(qc, CH)])
                    ps = ps3.tile([P, CH], F32, name="pso", tag="pso")
                    for ks in range(DP):
                        nc.tensor.matmul(ps[:], wo_sb[:, ks, ts(i, P)],
                                         attn_all[:, ks, ts(qc, CH)],
                                         start=(ks == 0), stop=(ks == DP - 1))
                    t1 = tp3.tile([P, CH], F32, name="t1", tag="t1")
                    nc.vector.tensor_add(t1[:], ps[:], xo[:])
                    nc.vector.tensor_scalar_add(x2T[:, i, ts(qc, CH)], t1[:],
                                                bias_sb[:, 0 + i:1 + i])
                # LN2 for this chunk immediately; overlaps next chunk's matmuls
                ln_norm(lp3, lambda ks, qc=qc: x2T[:, ks, ts(qc, CH)],
                        h2T[:, :, ts(qc, CH)])
        sATT.close()
        sWX.close()

        # ================= Phase 4: FFN + residual =================
        with ExitStack() as p4:
            w1p = p4.enter_context(tc.tile_pool(name="w1p", bufs=2))
            w2p = p4.enter_context(tc.tile_pool(name="w2p", bufs=2))
            rp = p4.enter_context(tc.tile_pool(name="rp", bufs=2))
            psF = p4.enter_context(tc.tile_pool(name="psF", bufs=4, space="PSUM"))
            psO = p4.enter_context(tc.tile_pool(name="psO", bufs=2, space="PSUM"))
            top = p4.enter_context(tc.tile_pool(name="top", bufs=4))
            for qc in range(TQ // CH):
                rT = rp.tile([P, FP, CH], BF16, name="rT", tag="rT")
                for fs in range(8):
                    w1c = w1p.tile([P, DP, CH], BF16, name="w1c", tag="w1c")
                    nc.sync.dma_start(w1c[:], w13[:, :, ts(fs, CH)])
                    for fj in range(4):
                        f = fs * 4 + fj
                        ps = psF.tile([P, CH], F32, name="psf", tag="psf")
                        for ks in range(DP):
                            nc.tensor.matmul(ps[:], w1c[:, ks, ts(fj, P)],
                                             h2T[:, ks, ts(qc, CH)],
                                             start=(ks == 0), stop=(ks == DP - 1))
                        nc.scalar.activation(rT[:, f], ps[:], AF.Relu,
                                             bias=bias_sb[:, 32 + f:33 + f])
                for i in range(DP):
                    w2i = w2p.tile([P, FP, P], BF16, name="w2i", tag="w2i")
                    nc.sync.dma_start(w2i[:], w23[:, :, ts(i, P)])
                    ps2 = psO.tile([P, CH], F32, name="ps2", tag="ps2")
                    for f in range(FP):
                        nc.tensor.matmul(ps2[:], w2i[:, f, :], rT[:, f],
                                         start=(f == 0), stop=(f == FP - 1))
                    t2 = top.tile([P, CH], F32, name="t2", tag="t2")
                    nc.vector.tensor_add(t2[:], ps2[:], x2T[:, i, ts(qc, CH)])
                    ot = top.tile([P, CH], F32, name="ot", tag="ot")
                    nc.vector.tensor_scalar_add(ot[:], t2[:], bias_sb[:, 8 + i:9 + i])
                    nc.sync.dma_start(out3[:, i, ts(qc, CH)], ot[:])
        sX2.close()

    nc.compile()
    return nc


def prepare_inputs(x, wq, wk, wv, wo, bo, w1, b1, w2, b2,
                   g_ln1, b_ln1, g_ln2, b_ln2):
    """Host-side sharding/prep. Returns list of 8 per-core input dicts."""
    f32 = np.float32
    bf = ml_dtypes.bfloat16
    x = np.asarray(x, f32)
    g1 = np.asarray(g_ln1, f32)
    b1n = np.asarray(b_ln1, f32)
    g2 = np.asarray(g_ln2, f32)
    b2n = np.asarray(b_ln2, f32)

    wq_e = np.ascontiguousarray((g1[:, None] * np.asarray(wq, f32)).astype(bf))
    wk_e = np.ascontiguousarray((g1[:, None] * np.asarray(wk, f32)).astype(bf))
    wv_e = np.ascontiguousarray((g1[:, None] * np.asarray(wv, f32)).astype(bf))
    wo_e = np.ascontiguousarray(np.asarray(wo, f32).astype(bf))
    w1_e = np.ascontiguousarray((g2[:, None] * np.asarray(w1, f32)).astype(bf))
    w2_e = np.ascontiguousarray(np.asarray(w2, f32).astype(bf))

    bq = b1n @ np.asarray(wq, f32)
    bk = b1n @ np.asarray(wk, f32)
    bv = b1n @ np.asarray(wv, f32)
    b1p = np.asarray(b1, f32) + b2n @ np.asarray(w1, f32)

    def pcol(v, n):  # [n*128] -> [128, n] partition-major
        return np.ascontiguousarray(np.asarray(v, f32).reshape(n, P).T)

    biases = np.zeros((P, 64), f32)
    biases[:, 0:8] = pcol(bo, 8)
    biases[:, 8:16] = pcol(b2, 8)
    biases[:, 16:24] = pcol(bq, 8)
    biases[:, 24:32] = pcol(bk, 8)
    biases[:, 32:64] = pcol(b1p, 32)
    bvr = np.ascontiguousarray(np.broadcast_to(bv[None, :], (P, D)))

    masks = {}
    for d in (0, 1):
        p = np.arange(P)[:, None]
        r = np.arange(64)[None, :]
        masks[d] = np.ascontiguousarray((p <= (2 * r + d)).astype(bf))

    in_maps = []
    for c in range(8):
        b, d = divmod(c, 2)
        in_maps.append(dict(
            xT=np.ascontiguousarray(x[b].T),
            xoT=np.ascontiguousarray(x[b, d::2].T),
            wq=wq_e, wk=wk_e, wv=wv_e, wo=wo_e, w1=w1_e, w2=w2_e,
            biases=biases, bvr=bvr, mk=masks[d],
        ))
    return in_maps


_NC = None
LAST_RESULTS = None


def kernel(**inputs):
    global _NC, LAST_RESULTS
    in_maps = prepare_inputs(**inputs)
    if _NC is None:
        _NC = build_nc()
    res = run_bass_kernel_spmd(_NC, in_maps, core_ids=list(range(8)))
    LAST_RESULTS = res
    out = np.empty((4, TKV, D), np.float32)
    for c in range(8):
        b, d = divmod(c, 2)
        out[b, d::2, :] = res.results[c]["outT"].T
    return out


if __name__ == "__main__":
    z = np.load("/root/problem/ref_cache.npz")
    inputs = {k: z[k] for k in z.files if k != "out"}
    out = kernel(**inputs)
    ref = z["out"]
    err = np.abs(out - ref)
    print("abs max err:", err.max(), "scale-rel:", err.max() / np.abs(ref).max())


# revision 43
# speedup vs baseline: 1.2983x; 1.0789x over previous
"""Trainium2 Bass kernel for a dense transformer block (nn_Block_31387620999284).

Sharding: 8 cores = 4 batches x 2 parity groups. Core c handles batch b=c//2
and the query tokens with sequence parity d=c%2 (positions d, d+2, ...). Every
core computes K/V for its batch's full 2048-token sequence (duplicated across
the pair), which removes all cross-core communication. Parity interleaving
makes the causal-attention work identical on every core, so a single NEFF runs
SPMD on all 8 cores with per-core input data only.

On-device layout is "transposed" throughout: [features on partitions, tokens on
free dim]. LayerNorm statistics are computed with ones-vector matmuls on the
tensor engine (partition-dim reduction), then broadcast back across partitions
with gpsimd.partition_broadcast. Matmuls run in bf16 (weights pre-cast on the
host, activations cast on the fly) with fp32 PSUM accumulation; softmax skips
the max-subtraction (scores for this block are bounded by ~3, exp is safe).
The softmax denominator rides along as a 65th ones-column in V, so attention
is exp + mask-multiply + one accumulating matmul chain per (head, q-block).
LN-affine params are folded into the weights/biases on the host.
"""

import sys

for _p in ("/opt/trn_rl_repo",):
    if _p not in sys.path:
        sys.path.append(_p)

import numpy as np
import ml_dtypes
from contextlib import ExitStack

import concourse.bass as bass
import concourse.tile as tile
from concourse import bacc, mybir
from concourse.bass import ts
from concourse.bass_utils import run_bass_kernel_spmd


def _install_ntff_hook():
    """The container's antenv stub lacks axon_hooks; provide it so tracing
    (BASS_TRACE=1) works instead of crashing on import."""
    try:
        import antenv.axon_hooks  # noqa: F401
        return
    except ImportError:
        pass
    try:
        import types
        import antenv
        mod = types.ModuleType("antenv.axon_hooks")
        mod._hook = None
        mod.set_axon_ntff_profile_hook = lambda h: setattr(mod, "_hook", h)
        mod.get_axon_ntff_profile_hook = lambda: mod._hook
        sys.modules["antenv.axon_hooks"] = mod
        antenv.axon_hooks = mod
        try:
            from trn_agent_boot.trn_boot import _ntff_profile_via_ctypes
            mod._hook = _ntff_profile_via_ctypes("/opt/axon/libaxon_pjrt.so")
        except Exception:
            pass
    except Exception:
        pass


_install_ntff_hook()

P = 128
D = 1024
TKV = 2048
TQ = 1024
F = 4096
H = 16
HD = 64
DP = D // P    # 8
FP = F // P    # 32
CH = 512       # token chunk / matmul free dim
QB = 512       # attention query block
NQB = TQ // QB # 2
NKT = TKV // P # 16 key tiles
EPS = 1e-5

F32 = mybir.dt.float32
BF16 = mybir.dt.bfloat16
AF = mybir.ActivationFunctionType


def build_nc():
    nc = bacc.Bacc("TRN2", target_bir_lowering=False, debug=False)

    xT = nc.dram_tensor("xT", [D, TKV], F32, kind="ExternalInput").ap()
    xoT = nc.dram_tensor("xoT", [D, TQ], F32, kind="ExternalInput").ap()
    wq = nc.dram_tensor("wq", [D, D], BF16, kind="ExternalInput").ap()
    wk = nc.dram_tensor("wk", [D, D], BF16, kind="ExternalInput").ap()
    wv = nc.dram_tensor("wv", [D, D], BF16, kind="ExternalInput").ap()
    wo = nc.dram_tensor("wo", [D, D], BF16, kind="ExternalInput").ap()
    w1 = nc.dram_tensor("w1", [D, F], BF16, kind="ExternalInput").ap()
    w2 = nc.dram_tensor("w2", [F, D], BF16, kind="ExternalInput").ap()
    # bias columns: bo 0:8 | b2 8:16 | bq 16:24 | bk 24:32 | b1' 32:64
    biases = nc.dram_tensor("biases", [P, 64], F32, kind="ExternalInput").ap()
    bvr = nc.dram_tensor("bvr", [P, D], F32, kind="ExternalInput").ap()
    mk = nc.dram_tensor("mk", [P, 2, 64], BF16, kind="ExternalInput").ap()
    outT = nc.dram_tensor("outT", [D, TQ], F32, kind="ExternalOutput").ap()

    xT3 = xT.rearrange("(o p) t -> p o t", p=P)
    xoT3 = xoT.rearrange("(o p) t -> p o t", p=P)
    out3 = outT.rearrange("(o p) t -> p o t", p=P)
    wq3 = wq.rearrange("(o p) m -> p o m", p=P)
    wk3 = wk.rearrange("(o p) m -> p o m", p=P)
    wv3 = wv.rearrange("(o p) m -> p o m", p=P)
    wo3 = wo.rearrange("(o p) m -> p o m", p=P)
    w13 = w1.rearrange("(o p) m -> p o m", p=P)
    w23 = w2.rearrange("(o p) m -> p o m", p=P)

    with tile.TileContext(nc) as tc, ExitStack() as ctx:
        consts = ctx.enter_context(tc.tile_pool(name="consts", bufs=1))
        bias_sb = consts.tile([P, 64], F32, name="bias_sb")
        nc.sync.dma_start(bias_sb[:], biases)
        ones_b16 = consts.tile([P, 1], BF16, name="ones_b16")
        nc.vector.memset(ones_b16[:], 1.0)
        ones_f32 = consts.tile([1, P], F32, name="ones_f32")
        nc.vector.memset(ones_f32[:], 1.0)
        eps_sb = consts.tile([P, 1], F32, name="eps_sb")
        nc.vector.memset(eps_sb[:], EPS)

        # ---- LayerNorm (transposed layout) ----
        def make_ln_pools(stack, pfx):
            return dict(
                sq=stack.enter_context(tc.tile_pool(name=pfx + "sq", bufs=2)),
                st=stack.enter_context(tc.tile_pool(name=pfx + "st", bufs=2, space="PSUM")),
                sm=stack.enter_context(tc.tile_pool(name=pfx + "sm", bufs=1)),
                rep=stack.enter_context(tc.tile_pool(name=pfx + "rep", bufs=1, space="PSUM")),
                rsb=stack.enter_context(tc.tile_pool(name=pfx + "rsb", bufs=2)),
            )

        def ln_norm(lp, get_src, hc):
            """get_src(ks) -> [P, CH] f32 AP; hc: [P, DP, CH] bf16 out.

            Casts x to bf16 into hc, computes mean/var from the bf16 values via
            ones-matmuls, then normalizes hc in place."""
            ps_su = lp["st"].tile([1, CH], F32, name="ps_su", tag="st")
            ps_sq = lp["st"].tile([1, CH], F32, name="ps_sq", tag="st")
            for ks in range(DP):
                src = get_src(ks)
                nc.scalar.copy(hc[:, ks], src)
                sq = lp["sq"].tile([P, CH], BF16, name="sq", tag="sq")
                nc.scalar.activation(sq[:], src, AF.Square)
                nc.tensor.matmul(ps_su[:], ones_b16[:], hc[:, ks],
                                 start=(ks == 0), stop=(ks == DP - 1))
                nc.tensor.matmul(ps_sq[:], ones_b16[:], sq[:],
                                 start=(ks == 0), stop=(ks == DP - 1))
            # r_mu = -mean; r_m2 -> var -> sd; replicate sd and -mu across
            # partitions on the PE, then 1/sd and -mu/sd at full lane width.
            r_mu = lp["sm"].tile([1, CH], F32, name="r_mu", tag="r_mu")
            nc.vector.tensor_scalar_mul(r_mu[:], ps_su[:], -1.0 / D)
            r_m2 = lp["sm"].tile([1, CH], F32, name="r_m2", tag="r_m2")
            nc.vector.tensor_scalar_mul(r_m2[:], ps_sq[:], 1.0 / D)
            mu2 = lp["sm"].tile([1, CH], F32, name="mu2", tag="mu2")
            nc.vector.tensor_mul(mu2[:], r_mu[:], r_mu[:])
            nc.vector.tensor_sub(r_m2[:], r_m2[:], mu2[:])
            nc.scalar.activation(r_m2[:], r_m2[:], AF.Sqrt, bias=eps_sb[0:1])
            repS = lp["rep"].tile([P, CH], F32, name="repS", tag="repS")
            nc.tensor.matmul(repS[:], ones_f32[:], r_m2[:], start=True, stop=True)
            repM = lp["rep"].tile([P, CH], F32, name="repM", tag="repM")
            nc.tensor.matmul(repM[:], ones_f32[:], r_mu[:], start=True, stop=True)
            repA = lp["rsb"].tile([P, CH], F32, name="repA", tag="repA")
            nc.vector.reciprocal(repA[:], repS[:])
            repB = lp["rsb"].tile([P, CH], F32, name="repB", tag="repB")
            nc.vector.tensor_mul(repB[:], repM[:], repA[:])
            for ks in range(DP):
                nc.vector.tensor_mul(hc[:, ks], hc[:, ks], repA[:])
                nc.vector.tensor_add(hc[:, ks], hc[:, ks], repB[:])

        # Persistent K/V/Q for attention (phases 1-2).
        sKVQ = ExitStack()
        kvqp = sKVQ.enter_context(tc.tile_pool(name="kvqp", bufs=1))
        KT_all = kvqp.tile([P, DP, TKV], BF16, name="KT_all")
        V_all = kvqp.tile([P, NKT, H, HD + 1], BF16, name="V_all")
        QT_all = kvqp.tile([P, DP, TQ], BF16, name="QT_all")

        # Pools that must outlive phase transitions sit on the right side so
        # their DMAs never alias freed left-side addresses (no false deps).
        sWX = ExitStack()
        mskp = sWX.enter_context(tc.tile_pool(name="mskp", bufs=1, side="right"))
        mask_sb = mskp.tile([P, 2, 64], BF16, name="mask_sb")
        nc.sync.dma_start(mask_sb[:], mk)
        wop = sWX.enter_context(tc.tile_pool(name="wop", bufs=1, side="right"))
        wo_sb = wop.tile([P, DP, D], BF16, name="wo_sb")
        xop = sWX.enter_context(tc.tile_pool(name="xop", bufs=3, side="right"))

        # ================= Phase 1: LN1 + Q/K/V projections =================
        with ExitStack() as p1:
            lp1 = make_ln_pools(p1, "l1")
            xcp = p1.enter_context(tc.tile_pool(name="xcp", bufs=4))
            hcp = p1.enter_context(tc.tile_pool(name="hcp", bufs=2))
            mmp = p1.enter_context(tc.tile_pool(name="mmp1", bufs=3, space="PSUM"))
            bvp = p1.enter_context(tc.tile_pool(name="bvp", bufs=1))
            wkvp = p1.enter_context(tc.tile_pool(name="wkvp", bufs=1))
            wqp = p1.enter_context(tc.tile_pool(name="wqp", bufs=1, side="right"))

            # chunk list: 4 KV chunks then 2 Q chunks, software-pipelined so the
            # LN chain of chunk i+1 overlaps the projection matmuls of chunk i.
            chunks = [("kv", c) for c in range(TKV // CH)] + \
                     [("q", c) for c in range(TQ // CH)]
            hcs = {}

            def load_ln(idx):
                kind, c = chunks[idx]
                src3 = xT3 if kind == "kv" else xoT3

                def get_src(ks, src3=src3, c=c):
                    xk = xcp.tile([P, CH], F32, name="xk", tag="xk")
                    nc.sync.dma_start(xk[:], src3[:, ks, ts(c, CH)])
                    return xk[:]

                hc = hcp.tile([P, DP, CH], BF16, name="hc", tag="hc")
                ln_norm(lp1, get_src, hc)
                hcs[idx] = hc

            load_ln(0)

            # weight/bias loads traced after the first chunk's LN so the PE can
            # start on stats immediately; DMAs overlap the LN chain.
            nc.vector.memset(V_all[:, :, :, HD:HD + 1], 1.0)
            bvr_sb = bvp.tile([P, D], F32, name="bvr_sb")
            nc.sync.dma_start(bvr_sb[:], bvr)
            wk_sb = wkvp.tile([P, DP, D], BF16, name="wk_sb")
            nc.sync.dma_start(wk_sb[:], wk3)
            wv_sb = wkvp.tile([P, DP, D], BF16, name="wv_sb")
            nc.sync.dma_start(wv_sb[:], wv3)
            wq_sb = wqp.tile([P, DP, D], BF16, name="wq_sb")
            nc.sync.dma_start(wq_sb[:], wq3)
            nc.sync.dma_start(wo_sb[:], wo3)

            for idx, (kind, c) in enumerate(chunks):
                if idx + 1 < len(chunks):
                    load_ln(idx + 1)
                hc = hcs.pop(idx)
                if kind == "kv":
                    for hp in range(DP):
                        ps = mmp.tile([P, CH], F32, name="psk", tag="mm1")
                        for ks in range(DP):
                            nc.tensor.matmul(ps[:], wk_sb[:, ks, ts(hp, P)], hc[:, ks],
                                             start=(ks == 0), stop=(ks == DP - 1))
                        nc.vector.tensor_scalar_add(KT_all[:, hp, ts(c, CH)], ps[:],
                                                    bias_sb[:, 24 + hp:25 + hp])
                    for dc in range(2):
                        for st in range(4):
                            ps = mmp.tile([P, CH], F32, name="psv", tag="mm1")
                            for ks in range(DP):
                                nc.tensor.matmul(ps[:], hc[:, ks, ts(st, P)],
                                                 wv_sb[:, ks, ts(dc, CH)],
                                                 start=(ks == 0), stop=(ks == DP - 1))
                            vdst = V_all[:, c * 4 + st, dc * 8:dc * 8 + 8, 0:HD]
                            nc.vector.tensor_add(
                                vdst,
                                ps[:].rearrange("p (h d) -> p h d", h=8),
                                bvr_sb[:, ts(dc, CH)].rearrange("p (h d) -> p h d", h=8))
                else:
                    for hp in range(DP):
                        ps = mmp.tile([P, CH], F32, name="psq", tag="mm1")
                        for ks in range(DP):
                            nc.tensor.matmul(ps[:], wq_sb[:, ks, ts(hp, P)], hc[:, ks],
                                             start=(ks == 0), stop=(ks == DP - 1))
                        nc.vector.tensor_scalar_add(QT_all[:, hp, ts(c, CH)], ps[:],
                                                    bias_sb[:, 16 + hp:17 + hp])

        # ================= Phase 2: causal attention =================
        sATT = ExitStack()
        attp = sATT.enter_context(tc.tile_pool(name="attp", bufs=1, side="right"))
        attn_all = attp.tile([P, DP, TQ], BF16, name="attn_all")
        with ExitStack() as p2:
            psS = p2.enter_context(tc.tile_pool(name="psS", bufs=2, space="PSUM"))
            psAV = p2.enter_context(tc.tile_pool(name="psAV", bufs=4, space="PSUM"))
            weip = p2.enter_context(tc.tile_pool(name="weip", bufs=4))
            smal = p2.enter_context(tc.tile_pool(name="smal", bufs=4))

            scale = float(HD) ** -0.5
            pend = []  # deferred softmax-denominator sections

            def flush_den():
                # Runs the denominator/normalize tail for a finished (t, hp)
                # off the PE critical path: by the time this is traced, the
                # next head-pair's score pipeline is already rolling.
                for t, hp, pavs in pend:
                    for l in range(2):
                        pb = 64 * l
                        pav = pavs[l]
                        den = smal.tile([1, QB], F32, name="den", tag="den")
                        nc.scalar.copy(den[:], pav[64:65, :])
                        # replicate 1/den across partitions: PE replicates den
                        # into pav rows 64:128, DVE reciprocal pulls it to SBUF
                        nc.tensor.matmul(pav[64:128, :], ones_f32[:, 0:64],
                                         den[:], start=True, stop=True)
                        rec = smal.tile([64, QB], F32, name="rec", tag="rec")
                        nc.vector.reciprocal(rec[:], pav[64:128, :])
                        nc.vector.tensor_mul(attn_all[pb:pb + 64, hp, ts(t, QB)],
                                             pav[0:64, :], rec[:])
                pend.clear()

            for t in range(NQB):           # q-blocks outermost: lets phase 3's
                nkt = 8 * (t + 1)          # first chunk start after t=0 is done
                for hp in range(DP):
                    pavs = [psAV.tile([P, QB], F32, name=f"pav{l}", tag="pav")
                            for l in range(2)]
                    weis = {}

                    # Causal structure: for q-block t, key tile kt = 8*t + j is
                    # "diagonal": columns < 64*j are fully masked (skipped
                    # entirely), columns [64j, 64j+64) need the (j-independent)
                    # stride-2 mask, the rest are fully allowed.
                    def col0(kt, t=t):
                        return 64 * (kt - 8 * t) if kt >= 8 * t else 0

                    def scores2(kt, t=t, hp=hp, weis=weis):
                        o = col0(kt)
                        ps2 = psS.tile([P, 2, QB], F32, name="pss2", tag="pss2")
                        for l in range(2):
                            pb = 64 * l
                            nc.tensor.matmul(ps2[:, l, o:],
                                             KT_all[pb:pb + 64, hp, ts(kt, P)],
                                             QT_all[pb:pb + 64, hp,
                                                    t * QB + o:(t + 1) * QB],
                                             start=True, stop=True)
                        wei2 = weip.tile([P, 2, QB], BF16, name="wei2", tag="wei2")
                        nc.scalar.activation(wei2[:, :, o:], ps2[:, :, o:], AF.Exp,
                                             scale=scale)
                        if kt >= 8 * t:
                            nc.vector.tensor_mul(wei2[:, :, o:o + 64],
                                                 wei2[:, :, o:o + 64], mask_sb[:])
                        weis[kt] = wei2

                    scores2(0)
                    for kt in range(nkt):
                        if kt + 1 < nkt:
                            scores2(kt + 1)
                        o = col0(kt)
                        wei2 = weis.pop(kt)
                        for l in range(2):
                            nc.tensor.matmul(pavs[l][0:65, o:],
                                             V_all[:, kt, 2 * hp + l, :],
                                             wei2[:, l, o:],
                                             start=(kt == 0), stop=(kt == nkt - 1))
                        if kt == 0:
                            flush_den()
                    pend.append((t, hp, pavs))
            flush_den()
        sKVQ.close()

        # ================= Phase 3: output proj + residual + LN2 =================
        sX2 = ExitStack()
        x2p = sX2.enter_context(tc.tile_pool(name="x2p", bufs=1))
        x2T = x2p.tile([P, DP, TQ], F32, name="x2T")
        h2T = x2p.tile([P, DP, TQ], BF16, name="h2T")
        with ExitStack() as p3:
            lp3 = make_ln_pools(p3, "l3")
            ps3 = p3.enter_context(tc.tile_pool(name="ps3", bufs=3, space="PSUM"))
            tp3 = p3.enter_context(tc.tile_pool(name="tp3", bufs=4))
            for qc in range(TQ // CH):
                for i in range(DP):
                    xo = xop.tile([P, CH], F32, name="xo", tag="xo")
                    nc.sync.dma_start(xo[:], xoT3[:, i, ts(qc, CH)])
                    ps = ps3.tile([P, CH], F32, name="pso", tag="pso")
                    for ks in range(DP):
                        nc.tensor.matmul(ps[:], wo_sb[:, ks, ts(i, P)],
                                         attn_all[:, ks, ts(qc, CH)],
                                         start=(ks == 0), stop=(ks == DP - 1))
                    t1 = tp3.tile([P, CH], F32, name="t1", tag="t1")
                    nc.vector.tensor_add(t1[:], ps[:], xo[:])
                    nc.vector.tensor_scalar_add(x2T[:, i, ts(qc, CH)], t1[:],
                                                bias_sb[:, 0 + i:1 + i])
                # LN2 for this chunk immediately; overlaps next chunk's matmuls
                ln_norm(lp3, lambda ks, qc=qc: x2T[:, ks, ts(qc, CH)],
                        h2T[:, :, ts(qc, CH)])
        sATT.close()
        sWX.close()

        # ================= Phase 4: FFN + residual =================
        with ExitStack() as p4:
            w1p = p4.enter_context(tc.tile_pool(name="w1p", bufs=2, side="right"))
            w2p = p4.enter_context(tc.tile_pool(name="w2p", bufs=2, side="right"))
            rp = p4.enter_context(tc.tile_pool(name="rp", bufs=2))
            psF = p4.enter_context(tc.tile_pool(name="psF", bufs=4, space="PSUM"))
            psO = p4.enter_context(tc.tile_pool(name="psO", bufs=2, space="PSUM"))
            top = p4.enter_context(tc.tile_pool(name="top", bufs=4))
            for qc in range(TQ // CH):
                rT = rp.tile([P, FP, CH], BF16, name="rT", tag="rT")
                for fs in range(8):
                    w1c = w1p.tile([P, DP, CH], BF16, name="w1c", tag="w1c")
                    nc.sync.dma_start(w1c[:], w13[:, :, ts(fs, CH)])
                    for fj in range(4):
                        f = fs * 4 + fj
                        ps = psF.tile([P, CH], F32, name="psf", tag="psf")
                        for ks in range(DP):
                            nc.tensor.matmul(ps[:], w1c[:, ks, ts(fj, P)],
                                             h2T[:, ks, ts(qc, CH)],
                                             start=(ks == 0), stop=(ks == DP - 1))
                        nc.scalar.activation(rT[:, f], ps[:], AF.Relu,
                                             bias=bias_sb[:, 32 + f:33 + f])
                for i in range(DP):
                    w2i = w2p.tile([P, FP, P], BF16, name="w2i", tag="w2i")
                    nc.sync.dma_start(w2i[:], w23[:, :, ts(i, P)])
                    ps2 = psO.tile([P, CH], F32, name="ps2", tag="ps2")
                    for f in range(FP):
                        nc.tensor.matmul(ps2[:], w2i[:, f, :], rT[:, f],
                                         start=(f == 0), stop=(f == FP - 1))
                    t2 = top.tile([P, CH], F32, name="t2", tag="t2")
                    nc.vector.tensor_add(t2[:], ps2[:], x2T[:, i, ts(qc, CH)])
                    ot = top.tile([P, CH], F32, name="ot", tag="ot")
                    nc.vector.tensor_scalar_add(ot[:], t2[:], bias_sb[:, 8 + i:9 + i])
                    nc.sync.dma_start(out3[:, i, ts(qc, CH)], ot[:])
        sX2.close()

    nc.compile()
    return nc


def prepare_inputs(x, wq, wk, wv, wo, bo, w1, b1, w2, b2,
                   g_ln1, b_ln1, g_ln2, b_ln2):
    """Host-side sharding/prep. Returns list of 8 per-core input dicts."""
    f32 = np.float32
    bf = ml_dtypes.bfloat16
    x = np.asarray(x, f32)
    g1 = np.asarray(g_ln1, f32)
    b1n = np.asarray(b_ln1, f32)
    g2 = np.asarray(g_ln2, f32)
    b2n = np.asarray(b_ln2, f32)

    wq_e = np.ascontiguousarray((g1[:, None] * np.asarray(wq, f32)).astype(bf))
    wk_e = np.ascontiguousarray((g1[:, None] * np.asarray(wk, f32)).astype(bf))
    wv_e = np.ascontiguousarray((g1[:, None] * np.asarray(wv, f32)).astype(bf))
    wo_e = np.ascontiguousarray(np.asarray(wo, f32).astype(bf))
    w1_e = np.ascontiguousarray((g2[:, None] * np.asarray(w1, f32)).astype(bf))
    w2_e = np.ascontiguousarray(np.asarray(w2, f32).astype(bf))

    bq = b1n @ np.asarray(wq, f32)
    bk = b1n @ np.asarray(wk, f32)
    bv = b1n @ np.asarray(wv, f32)
    b1p = np.asarray(b1, f32) + b2n @ np.asarray(w1, f32)

    def pcol(v, n):  # [n*128] -> [128, n] partition-major
        return np.ascontiguousarray(np.asarray(v, f32).reshape(n, P).T)

    biases = np.zeros((P, 64), f32)
    biases[:, 0:8] = pcol(bo, 8)
    biases[:, 8:16] = pcol(b2, 8)
    biases[:, 16:24] = pcol(bq, 8)
    biases[:, 24:32] = pcol(bk, 8)
    biases[:, 32:64] = pcol(b1p, 32)
    bvr = np.ascontiguousarray(np.broadcast_to(bv[None, :], (P, D)))

    masks = {}
    for d in (0, 1):
        p = np.arange(P)[:, None]
        r = np.arange(64)[None, :]
        m = (p <= (2 * r + d)).astype(bf)
        masks[d] = np.ascontiguousarray(np.broadcast_to(m[:, None, :], (P, 2, 64)))

    in_maps = []
    for c in range(8):
        b, d = divmod(c, 2)
        in_maps.append(dict(
            xT=np.ascontiguousarray(x[b].T),
            xoT=np.ascontiguousarray(x[b, d::2].T),
            wq=wq_e, wk=wk_e, wv=wv_e, wo=wo_e, w1=w1_e, w2=w2_e,
            biases=biases, bvr=bvr, mk=masks[d],
        ))
    return in_maps


_NC = None
LAST_RESULTS = None


def kernel(**inputs):
    global _NC, LAST_RESULTS
    in_maps = prepare_inputs(**inputs)
    if _NC is None:
        _NC = build_nc()
    res = run_bass_kernel_spmd(_NC, in_maps, core_ids=list(range(8)))
    LAST_RESULTS = res
    out = np.empty((4, TKV, D), np.float32)
    for c in range(8):
        b, d = divmod(c, 2)
        out[b, d::2, :] = res.results[c]["outT"].T
    return out


if __name__ == "__main__":
    z = np.load("/root/problem/ref_cache.npz")
    inputs = {k: z[k] for k in z.files if k != "out"}
    out = kernel(**inputs)
    ref = z["out"]
    err = np.abs(out - ref)
    print("abs max err:", err.max(), "scale-rel:", err.max() / np.abs(ref).max())
